# revision 55
# baseline (speedup 1.0000x reference)
"""Trainium2 Bass kernel for nn_BiasAttention (B=8, N=2048, C=256, H=8).

Sharding: data-parallel over batch B across the 8 NeuronCores (one batch
element per core).  Weights and atten_bias are replicated to every core.

Per-core dataflow (everything kept "transposed" so the contraction dim is
always on SBUF partitions):
  xT   = x^T                              [C, N]     (PE transpose)
  qT   = ALPHA * (wq rows @ xT)           [256, N]   (heads at partitions 32h)
  kT   = wk rows @ xT                     [256, N]
  v    = x @ wv^T                         [N, 256]   (lhsT for attnv)
  sigT = (1 + tanh(bias/2))^T = 2*sigmoid(bias)^T    bf16 (PE transp + ACT
                                                     tanh + Pool +1)
  per head-group hg (4 heads), query block nb (512), key tile mt (128):
    S^T[m,(h,n)] = kT.T @ qT   4 heads via row-packed K=32 matmuls -> PSUM
    then one of three extraction paths (load-balancing DVE vs ACT, both of
    which are the only engines with a PSUM port):
      PUN  (9/16 mts): one custom-DVE op computes the bf16 BIT PATTERN of
           exp(x) via a Schraudolph magic-bias round in float arithmetic,
           written as int16 -> e_t. No ACT exp needed.
      PSI  (3/16): ACT copies S^T to SBUF bf16; DVE multiplies by sigT at
           2x (all-bf16); ACT exp.
      SIG  (4/16): DVE tensor_tensor (PSUM, 1x) multiplies by sigT; ACT exp.
    outT[d,n] += v^T E      4 heads via col-packed M=33 matmuls (|ones row
    gives the softmax denominator Z in the same matmul)
  out = outT / Z             (fast reciprocal + partition-broadcast DMA)
  yT = wproj^T @ out         then + b_proj, PE transpose, DMA out

The score scale, the sigmoid's 1/2, and the pun's 128*log2(e) are all
folded into ALPHA applied once on qT; ACT exp uses scale=ln2/128.
"""

import math

import numpy as np

B, N, C, H = 8, 2048, 256, 8
D = C // H  # 32
NCORES = 8
HG = 2  # head groups of 4
NB = N // 512  # 4 query blocks
MT = N // 128  # 16 key tiles

LOG2E = 1.4426950408889634
LN2 = 0.6931471805599453
MAGIC = 12582912.0            # 1.5 * 2^23
CPUN = 0.0575                 # RMS-optimal Schraudolph shift
OFF = 128.0 * (127.0 - CPUN)
SCALE = D ** -0.5
ALPHA = 64.0 * LOG2E * SCALE  # folds score scale, sigma/2, pun 128*log2e
EXPSCALE = LN2 / 128.0

# Path per (mt, hp) slot: "pun" (custom DVE op, no ACT), "psi" (ACT copy +
# 2x DVE TT + ACT exp), "sig" (1x DVE TT + ACT exp). Chosen to balance
# DVE vs ACT engine time; pun fraction bounded by accuracy headroom.
PSI_SLOTS = frozenset({(1, 0), (1, 1), (3, 0), (3, 1), (5, 0), (5, 1),
                       (7, 0), (7, 1), (9, 0)})
SIG_SLOTS = frozenset({(15, 0)})
ALL_SLOTS = frozenset((mt, hp) for mt in range(MT) for hp in range(2))
PUN_SLOTS = ALL_SLOTS - PSI_SLOTS - SIG_SLOTS

_cache = {}


def _register_pun():
    import concourse.dve_ops as dve_ops
    from concourse.dve_spec import Spec, Src0, Src1, C0, C1, lower
    from concourse.dve_uop import DveOpSpec

    for op in dve_ops.OPS:
        if op.name == "PUN16":
            return op
    body = (Src0 * Src1 + C0) - C1
    ref = lambda in0, in1, s0, s1, imm2: (in0 * in1 + s0) - s1
    op = dve_ops.DveOp("PUN16", Spec(body=body, reference=ref),
                       subdim=False, uops_sha={})
    for ver in ("v3", "v4"):
        spec_c = DveOpSpec(name=op.name, opcode=0,
                           uops=lower(op.spec, ver=ver), rd1_en=True)
        op.uops_sha[ver] = spec_c.sha(ver)
    dve_ops.OPS.append(op)
    dve_ops.CUSTOM_DVE_SPECS[op.name] = op.spec
    dve_ops._SUB_OPCODE_FOR_NAME[op.name] = (
        dve_ops._CUSTOM_DVE_ROW_BASE + len(dve_ops.OPS) - 1)
    return op


def _build_module(reps=1, mode="full", psi_slots=None, sig_slots=None):
    if psi_slots is None:
        psi_slots = PSI_SLOTS
    if sig_slots is None:
        sig_slots = SIG_SLOTS
    pun_slots = ALL_SLOTS - frozenset(psi_slots) - frozenset(sig_slots)
    import concourse.bacc as bacc
    import concourse.mybir as mybir
    import concourse.tile as tile
    from concourse.bass import ds, ts
    from concourse.masks import make_identity

    PUN16 = _register_pun()

    f32 = mybir.dt.float32
    bf16 = mybir.dt.bfloat16
    i16 = mybir.dt.int16
    AF = mybir.ActivationFunctionType
    MUL = mybir.AluOpType.mult
    ADD = mybir.AluOpType.add

    nc = bacc.Bacc("TRN2", target_bir_lowering=False, debug=False,
                   num_devices=NCORES)

    x_d = nc.dram_tensor("x", [N, C], f32, kind="ExternalInput")
    bias_d = nc.dram_tensor("atten_bias", [N, N], f32, kind="ExternalInput")
    wqkv_d = nc.dram_tensor("w_qkv", [3 * C, C], f32, kind="ExternalInput")
    wproj_d = nc.dram_tensor("w_proj", [C, C], f32, kind="ExternalInput")
    bproj_d = nc.dram_tensor("b_proj", [C], f32, kind="ExternalInput")
    y_d = nc.dram_tensor("y", [N, C], f32, kind="ExternalOutput")

    with tile.TileContext(nc) as tc:
      for _rep in range(reps):
            with (
                tc.tile_pool(name="const", bufs=1) as const,
                tc.tile_pool(name="big", bufs=1) as big,
                tc.tile_pool(name="epool", bufs=3) as epool,
                tc.tile_pool(name="spool", bufs=3) as spool,
                tc.tile_pool(name="sigpool", bufs=2) as sigpool,
                tc.tile_pool(name="otn", bufs=1) as otn,
                tc.tile_pool(name="zstage", bufs=1) as zstage,
                tc.tile_pool(name="rzpool", bufs=2) as rzpool,
                tc.tile_pool(name="ytpool", bufs=2) as ytpool,
                tc.tile_pool(name="ystage", bufs=3) as ystage,
                tc.tile_pool(name="bstage", bufs=2) as bstage,
                tc.tile_pool(name="dpool", bufs=2, space="DRAM") as dpool,
                tc.tile_pool(name="aux", bufs=2, space="PSUM") as aux,
                tc.tile_pool(name="scps", bufs=2, space="PSUM") as scps,
                tc.tile_pool(name="outps", bufs=2, space="PSUM") as outps,
            ):
                ident = const.tile([128, 128], f32)
                make_identity(nc, ident)
                # Z-broadcast selector: out[m,n] = Z(row 32) for m<64,
                # Z(row 96) for m>=64, via one matmul per b-half.
                zsel = const.tile([128, 128], f32)
                nc.vector.memset(zsel, 0.0)
                nc.vector.memset(zsel[32:33, 0:64], 1.0)
                nc.vector.memset(zsel[96:97, 64:128], 1.0)
                bproj_sb = const.tile([128, 2], f32)
                nc.sync.dma_start(bproj_sb, bproj_d[:].rearrange("(j p) -> p j", p=128))

                wqkvT = const.tile([128, 2, 768], bf16)   # [c, cc, o]
                wprojT = const.tile([128, 2, 256], bf16)  # [c, cc, j]
                wprojP = const.tile([128, 4, 256], bf16)  # permuted for aug layout
                qT = big.tile([128, HG, N], bf16)         # [32h+d, hg, n]
                kT = big.tile([128, HG, N], bf16)
                v_aug = big.tile([128, MT, 8, 33], bf16)  # [m, mt, h, (d|1)]
                outTn = otn.tile([128, 4, N], bf16)       # [aug-c, 2hg+b, n]
                sig_tiles = {}                            # nbi -> [m, mt, 512]

                # ---------------- P0: weights + x transpose + qkv ----------------
                def prologue(stage, xtp):
                    xT = xtp.tile([128, 2, N], bf16)      # [c, cc, n]
                    # w_qkv^T and w_proj^T
                    for wt, (wd, rows) in enumerate([(wqkv_d, 6), (wproj_d, 2)]):
                        dest = wqkvT if wt == 0 else wprojT
                        for ot in range(rows):
                            wst = stage.tile([128, 256], f32, tag="wst")
                            nc.sync.dma_start(wst, wd[ts(ot, 128), :])
                            pst = aux.tile([128, 512], f32, tag="aux")
                            for cc in range(2):
                                nc.tensor.transpose(pst[:, ts(cc, 128)],
                                                    wst[:, ts(cc, 128)], ident)
                            for cc in range(2):
                                nc.scalar.copy(dest[:, cc, ts(ot, 128)],
                                               pst[:, ts(cc, 128)])
                    # x^T  (one contiguous 512-row DMA per quarter)
                    for ntq in range(4):
                        xst = stage.tile([128, 4, 256], f32, tag="xst")
                        nc.sync.dma_start(
                            xst, x_d[ts(ntq, 512), :].rearrange(
                                "(a p) c -> p a c", p=128))
                        for a in range(4):
                            nt = ntq * 4 + a
                            pst = aux.tile([128, 512], f32, tag="aux")
                            for cc in range(2):
                                nc.tensor.transpose(pst[:, ts(cc, 128)],
                                                    xst[:, a, ts(cc, 128)],
                                                    ident)
                            for cc in range(2):
                                nc.vector.tensor_copy(xT[:, cc, ts(nt, 128)],
                                                      pst[:, ts(cc, 128)])

                    # qT, kT  (o tiles 0,1 -> q ; 2,3 -> k); q scaled by ALPHA.
                    # nb-major emission so attention on (nb=0, hg=0) can start
                    # as soon as its two projections land.
                    def qk(og, nb):
                        dest = qT if og < 2 else kT
                        hg = og % 2
                        ps = aux.tile([128, 512], f32, tag="aux")
                        for cc in range(2):
                            nc.tensor.matmul(ps, wqkvT[:, cc, ts(og, 128)],
                                             xT[:, cc, ts(nb, 512)],
                                             start=(cc == 0), stop=(cc == 1))
                        if og < 2:
                            nc.vector.tensor_scalar_mul(
                                dest[:, hg, ts(nb, 512)], ps, ALPHA)
                        else:
                            nc.vector.tensor_copy(dest[:, hg, ts(nb, 512)], ps)

                    def v_tile(mt):
                        ps = aux.tile([128, 512], f32, tag="aux")
                        for cc in range(2):
                            nc.tensor.matmul(ps[:, :256], xT[:, cc, ts(mt, 128)],
                                             wqkvT[:, cc, 512:768],
                                             start=(cc == 0), stop=(cc == 1))
                        nc.vector.tensor_copy(
                            v_aug[:, mt, :, 0:32],
                            ps[:, :256].rearrange("p (h d) -> p h d", h=8))

                    nc.vector.memset(v_aug[:, :, :, 32:33], 1.0)
                    qk(0, 0)
                    qk(2, 0)
                    qk(1, 0)
                    qk(3, 0)
                    for mt in range(MT):
                        v_tile(mt)
                    for nb in range(1, NB):
                        for og in [0, 2, 1, 3]:
                            qk(og, nb)
                    # permuted w_proj^T matching the [out|Z] interleaved layout:
                    # chunk cc2 = 2*hg + b holds head (4hg+2b) at rows 0-31 and
                    # head (4hg+2b+1) at rows 64-95; Z rows get zero weights.
                    nc.vector.memset(wprojP, 0.0)
                    for hg in range(HG):
                        for b in range(2):
                            nc.sync.dma_start(wprojP[0:32, 2 * hg + b, :],
                                              wprojT[64 * b:64 * b + 32, hg, :])
                            nc.sync.dma_start(wprojP[64:96, 2 * hg + b, :],
                                              wprojT[64 * b + 32:64 * b + 64, hg, :])

                with tc.tile_pool(name="stage", bufs=4) as stage, \
                     tc.tile_pool(name="xtp", bufs=1) as xtp:
                    prologue(stage, xtp)

                # ---------------- P1+P2: attention, bias sigmoid JIT -------------
                def bias_unit(sigT, bst, nt4, mq, on_dve):
                    pst = aux.tile([128, 512], f32, tag="aux")
                    for j in range(4):
                        nc.tensor.transpose(
                            pst[:, ts(j, 128)],
                            bst[:, mq, ts(j, 128)], ident)
                    sg = sigT[:, mq * 4:(mq + 1) * 4, ts(nt4, 128)]
                    nc.scalar.activation(
                        sg, pst.rearrange("p (j f) -> p j f", j=4),
                        AF.Tanh, scale=0.5)
                    if on_dve:
                        nc.vector.tensor_scalar(sg, sg, 1.0, 1.0, MUL, ADD)
                    else:
                        nc.gpsimd.tensor_scalar(sg, sg, 1.0, 1.0, MUL, ADD)

                def bias_block(nbi, on_dve=False):
                    # produce sig tile (=2*sigmoid^T) for bias rows
                    # [512*nbi, 512*nbi+512); one fully-contiguous 1MB DMA
                    # per 128 bias rows (cheap SP dispatch), then per-unit
                    # closures the caller spreads between attention steps.
                    sigT = sigpool.tile([128, MT, 512], bf16, tag="sig")
                    sig_tiles[nbi] = sigT
                    units = []
                    for nt4 in range(4):
                        nt = 4 * nbi + nt4

                        def load(nt=nt):
                            bst = bstage.tile([128, 4, 512], f32, tag="bst")
                            nc.sync.dma_start(
                                bst, bias_d[ts(nt, 128), :].rearrange(
                                    "p (a f) -> p a f", a=4))
                            return bst

                        units.append((load, nt4))
                    out = []
                    for load, nt4 in units:
                        def group(load=load, nt4=nt4):
                            bst = load()
                            for mq in range(4):
                                bias_unit(sigT, bst, nt4, mq, on_dve)
                        out.append(group)
                    return out

                LAG = 6  # j-steps between scores+mul and the matching attnv
                pending = []

                def drain(limit):
                    while len(pending) > limit:
                        pending.pop(0)()

                do_mul = mode != "scores"
                do_attnv = mode in ("full", "noexp")
                do_exp = mode in ("full",)
                for u in bias_block(0, on_dve=True):
                    u()
                bias_units = []
                for nb in range(NB):
                    if nb + 1 < NB:
                        bias_units = bias_block(nb + 1)
                    for hg in range(HG):
                        out_ab = [outps.tile([128, 512], f32, tag="o",
                                             name=f"oab{hg}{nb}{b}")
                                  for b in range(2)]
                        e_ts = [None] * 4

                        def attnv(mq, j, hg=hg, nb=nb, out_ab=out_ab, e_ts=e_ts):
                            mt = mq * 4 + j
                            first = mt == 0
                            last = mt == MT - 1
                            e_t = e_ts[mq % 4]
                            for h in range(4):
                                nc.tensor.matmul(
                                    out_ab[h // 2][64 * (h % 2):64 * (h % 2) + 33, :],
                                    v_aug[:, mt, hg * 4 + h, :],
                                    e_t[:, j, h],
                                    start=first, stop=last,
                                    tile_position=(0, 64 * (h % 2)))

                        def tail(hg=hg, nb=nb, out_ab=out_ab):
                            # stage out+Z, broadcast Z across partitions with a
                            # select-matmul (no DRAM round-trip), then divide
                            # on the Pool engine (its only PSUM-free job).
                            st = zstage.tile([128, 2, 512], f32, tag="zst")
                            for b in range(2):
                                nc.scalar.copy(st[:, b, :], out_ab[b])
                            for b in range(2):
                                zb_ps = aux.tile([128, 512], f32, tag="aux")
                                nc.tensor.matmul(zb_ps, zsel, st[:, b, :],
                                                 start=True, stop=True)
                                zb = rzpool.tile([128, 512], f32, tag="rz")
                                nc.scalar.copy(zb, zb_ps)
                                rzb = rzpool.tile([128, 512], f32, tag="rz2")
                                nc.vector.reciprocal_approx_fast(rzb, zb)
                                nc.gpsimd.tensor_tensor(
                                    outTn[:, 2 * hg + b, ts(nb, 512)],
                                    st[:, b, :], rzb, MUL)
                            if hg == 1:
                                yts = []
                                for jt in range(2):
                                    pp = aux.tile([128, 512], f32, tag="aux")
                                    for cc2 in range(4):
                                        nc.tensor.matmul(
                                            pp, wprojP[:, cc2, ts(jt, 128)],
                                            outTn[:, cc2, ts(nb, 512)],
                                            start=(cc2 == 0), stop=(cc2 == 3))
                                    yt = ytpool.tile([128, 512], f32, tag="yt")
                                    nc.scalar.activation(
                                        yt, pp, AF.Identity,
                                        bias=bproj_sb[:, jt:jt + 1])
                                    yts.append(yt)
                                y_st = ystage.tile([128, 4, 256], f32,
                                                   tag="yst")
                                for k in range(4):
                                    yo = aux.tile([128, 512], f32, tag="aux")
                                    for jt in range(2):
                                        nc.tensor.transpose(
                                            yo[:, ts(jt, 128)],
                                            yts[jt][:, ts(k, 128)], ident)
                                    nc.scalar.copy(y_st[:, k, :], yo[:, :256])
                                nc.sync.dma_start(
                                    y_d[ts(nb, 512), :].rearrange(
                                        "(a p) c -> p a c", p=128), y_st)

                        def j_step(mq, j, e_t, hg=hg, nb=nb):
                            mt = mq * 4 + j
                            exp_hps = []
                            for hp in range(2):
                                punned = ((mt, hp) in pun_slots and do_exp
                                          and do_mul)
                                sc = scps.tile([128, 2, 512], f32, tag="s")
                                for hh in range(2):
                                    h = hp * 2 + hh
                                    nc.tensor.matmul(
                                        sc[:, hh, :],
                                        kT[32 * h:32 * (h + 1), hg, ts(mt, 128)],
                                        qT[32 * h:32 * (h + 1), hg, ts(nb, 512)],
                                        start=True, stop=True,
                                        tile_position=(32 * h, 0))
                                sig_bc = sig_tiles[nb][:, mt:mt + 1,
                                                       :].to_broadcast(
                                                           (128, 2, 512))
                                dst = e_t[:, j, 2 * hp:2 * hp + 2]
                                if not do_mul:
                                    nc.vector.tensor_copy(dst, sc)
                                elif punned:
                                    nc.vector._custom_dve(
                                        PUN16, out=dst.bitcast(i16),
                                        in0=sig_bc, in1=sc[:, :, :],
                                        s0=MAGIC + OFF, s1=MAGIC)
                                else:
                                    if (mt, hp) in psi_slots:
                                        s_sb = spool.tile([128, 2, 512], bf16,
                                                          tag="ssb")
                                        nc.scalar.copy(s_sb, sc)
                                        nc.vector.tensor_tensor(
                                            dst, s_sb, sig_bc, MUL)
                                    else:
                                        nc.vector.tensor_tensor(
                                            dst, sc, sig_bc, MUL)
                                    exp_hps.append(hp)
                            if do_exp and exp_hps:
                                if len(exp_hps) == 2:
                                    nc.scalar.activation(
                                        e_t[:, j], e_t[:, j], AF.Exp,
                                        scale=EXPSCALE)
                                else:
                                    hp = exp_hps[0]
                                    sl = e_t[:, j, 2 * hp:2 * hp + 2]
                                    nc.scalar.activation(
                                        sl, sl, AF.Exp, scale=EXPSCALE)

                        for mq in range(4):
                            e_t = epool.tile([128, 4, 4, 512], bf16, tag="e")
                            e_ts[mq % 4] = e_t
                            for j in range(4):
                                # drain one pending attnv BEFORE emitting the
                                # next scores so the PE never head-of-line
                                # blocks on the extract semaphore while attnv
                                # work is available.
                                if do_attnv:
                                    drain(LAG)
                                j_step(mq, j, e_t)
                                if do_attnv:
                                    pending.append(
                                        (lambda mq=mq, j=j, fn=attnv:
                                         fn(mq, j)))
                            if bias_units and (mq % 2 == 1 or hg == 1):
                                bias_units.pop(0)()
                        if do_attnv:
                            pending.append(tail)
                drain(0)

    nc.compile()
    return nc


def _get_module():
    if "nc" not in _cache:
        _cache["nc"] = _build_module()
    return _cache["nc"]


class _Runner:
    """Persistent jitted shard_map executor (mirrors bass2jax.run_bass_via_pjrt
    but keeps one jit cache entry so repeated calls don't recompile)."""

    def __init__(self, nc):
        import jax
        from jax.experimental.shard_map import shard_map
        from jax.sharding import Mesh, NamedSharding, PartitionSpec

        import concourse.mybir as mybir
        from concourse import bass2jax

        bass2jax.install_neuronx_cc_hook()
        assert nc.dbg_addr is None
        partition_name = (nc.partition_id_tensor.name
                          if nc.partition_id_tensor else None)
        in_names, out_names, out_avals, zero_outs = [], [], [], []
        for alloc in nc.m.functions[0].allocations:
            if not isinstance(alloc, mybir.MemoryLocationSet):
                continue
            name = alloc.memorylocations[0].name
            if alloc.kind == "ExternalInput":
                if name != partition_name:
                    in_names.append(name)
            elif alloc.kind == "ExternalOutput":
                out_names.append(name)
                shape = tuple(alloc.tensor_shape)
                dtype = mybir.dt.np(alloc.dtype)
                out_avals.append(jax.core.ShapedArray(shape, dtype))
                zero_outs.append(np.zeros(shape, dtype))
        self.in_names = in_names
        self.out_names = out_names
        self.out_avals = out_avals
        all_in = tuple(in_names) + tuple(out_names)
        if partition_name is not None:
            all_in = all_in + (partition_name,)

        def _body(*args):
            operands = list(args)
            if partition_name is not None:
                operands.append(bass2jax.partition_id_tensor())
            outs = bass2jax._bass_exec_p.bind(
                *operands,
                out_avals=tuple(out_avals),
                in_names=all_in,
                out_names=tuple(out_names),
                lowering_input_output_aliases=(),
                sim_require_finite=True,
                sim_require_nnan=True,
                nc=nc,
            )
            return tuple(outs)

        devices = jax.devices()[:NCORES]
        mesh = Mesh(np.asarray(devices), ("core",))
        nspec = len(in_names) + len(out_names)
        self._fn = jax.jit(
            shard_map(_body, mesh=mesh,
                      in_specs=(PartitionSpec("core"),) * nspec,
                      out_specs=(PartitionSpec("core"),) * len(out_names),
                      check_rep=False),
            keep_unused=True)
        self._sharding = NamedSharding(mesh, PartitionSpec("core"))
        self._jax = jax
        self._zero_dev = [
            jax.device_put(np.concatenate([z] * NCORES, axis=0), self._sharding)
            for z in zero_outs
        ]

    def put_inputs(self, in_maps):
        concat = [
            np.concatenate([np.asarray(m[nm]) for m in in_maps], axis=0)
            for nm in self.in_names
        ]
        return [self._jax.device_put(a, self._sharding) for a in concat]

    def run(self, dev_inputs):
        outs = self._fn(*dev_inputs, *self._zero_dev)
        self._jax.block_until_ready(outs)
        return outs


def _get_runner():
    if "runner" not in _cache:
        _cache["runner"] = _Runner(_get_module())
    return _cache["runner"]


def _make_in_maps(x, atten_bias, w_qkv, w_proj, b_proj):
    x = np.asarray(x, dtype=np.float32)
    atten_bias = np.ascontiguousarray(np.asarray(atten_bias, dtype=np.float32))
    w_qkv = np.ascontiguousarray(np.asarray(w_qkv, dtype=np.float32))
    w_proj = np.ascontiguousarray(np.asarray(w_proj, dtype=np.float32))
    b_proj = np.ascontiguousarray(np.asarray(b_proj, dtype=np.float32))
    return [
        {
            "x": np.ascontiguousarray(x[b]),
            "atten_bias": atten_bias,
            "w_qkv": w_qkv,
            "w_proj": w_proj,
            "b_proj": b_proj,
        }
        for b in range(B)
    ]


def kernel(x, atten_bias, w_qkv, w_proj, b_proj):
    runner = _get_runner()
    in_maps = _make_in_maps(x, atten_bias, w_qkv, w_proj, b_proj)
    dev = runner.put_inputs(in_maps)
    outs = runner.run(dev)
    y = np.asarray(outs[runner.out_names.index("y")])
    return y.reshape(B, N, C).astype(np.float32)


# revision 58
# speedup vs baseline: 1.3329x; 1.3329x over previous
"""Trainium2 Bass kernel for nn_BiasAttention (B=8, N=2048, C=256, H=8).

Sharding: data-parallel over batch B across the 8 NeuronCores (one batch
element per core).  Weights and atten_bias are replicated to every core.

Per-core dataflow (everything kept "transposed" so the contraction dim is
always on SBUF partitions):
  xT   = x^T                              [C, N]     (PE transpose)
  qT   = ALPHA * (wq rows @ xT)           [256, N]   (heads at partitions 32h)
  kT   = wk rows @ xT                     [256, N]
  v    = x @ wv^T                         [N, 256]   (lhsT for attnv)
  sigT = (1 + tanh(bias/2))^T = 2*sigmoid(bias)^T    bf16 (PE transp + ACT
                                                     tanh + Pool +1)
  per head-group hg (4 heads), query block nb (512), key tile mt (128):
    S^T[m,(h,n)] = kT.T @ qT   4 heads via row-packed K=32 matmuls -> PSUM
    then one of three extraction paths (load-balancing DVE vs ACT, both of
    which are the only engines with a PSUM port):
      PUN  (9/16 mts): one custom-DVE op computes the bf16 BIT PATTERN of
           exp(x) via a Schraudolph magic-bias round in float arithmetic,
           written as int16 -> e_t. No ACT exp needed.
      PSI  (3/16): ACT copies S^T to SBUF bf16; DVE multiplies by sigT at
           2x (all-bf16); ACT exp.
      SIG  (4/16): DVE tensor_tensor (PSUM, 1x) multiplies by sigT; ACT exp.
    outT[d,n] += v^T E      4 heads via col-packed M=33 matmuls (|ones row
    gives the softmax denominator Z in the same matmul)
  out = outT / Z             (fast reciprocal + partition-broadcast DMA)
  yT = wproj^T @ out         then + b_proj, PE transpose, DMA out

The score scale, the sigmoid's 1/2, and the pun's 128*log2(e) are all
folded into ALPHA applied once on qT; ACT exp uses scale=ln2/128.
"""

import math

import numpy as np

B, N, C, H = 8, 2048, 256, 8
D = C // H  # 32
NCORES = 8
HG = 2  # head groups of 4
NB = N // 512  # 4 query blocks
MT = N // 128  # 16 key tiles

LOG2E = 1.4426950408889634
LN2 = 0.6931471805599453
MAGIC = 12582912.0            # 1.5 * 2^23
CPUN = 0.0575                 # RMS-optimal Schraudolph shift
OFF = 128.0 * (127.0 - CPUN)
SCALE = D ** -0.5
ALPHA = 64.0 * LOG2E * SCALE  # folds score scale, sigma/2, pun 128*log2e
EXPSCALE = LN2 / 128.0

# Path per (mt, hp) slot: "pun" (custom DVE op, no ACT), "psi" (ACT copy +
# 2x DVE TT + ACT exp), "sig" (1x DVE TT + ACT exp). Chosen to balance
# DVE vs ACT engine time; pun fraction bounded by accuracy headroom.
PSI_SLOTS = frozenset({(1, 0), (1, 1), (3, 0), (3, 1), (5, 0), (5, 1),
                       (7, 0), (7, 1), (9, 0)})
SIG_SLOTS = frozenset({(15, 0)})
ALL_SLOTS = frozenset((mt, hp) for mt in range(MT) for hp in range(2))
PUN_SLOTS = ALL_SLOTS - PSI_SLOTS - SIG_SLOTS

_cache = {}


def _register_pun():
    import concourse.dve_ops as dve_ops
    from concourse.dve_spec import Spec, Src0, Src1, C0, C1, lower
    from concourse.dve_uop import DveOpSpec

    for op in dve_ops.OPS:
        if op.name == "PUN16":
            return op
    body = (Src0 * Src1 + C0) - C1
    ref = lambda in0, in1, s0, s1, imm2: (in0 * in1 + s0) - s1
    op = dve_ops.DveOp("PUN16", Spec(body=body, reference=ref),
                       subdim=False, uops_sha={})
    for ver in ("v3", "v4"):
        spec_c = DveOpSpec(name=op.name, opcode=0,
                           uops=lower(op.spec, ver=ver), rd1_en=True)
        op.uops_sha[ver] = spec_c.sha(ver)
    dve_ops.OPS.append(op)
    dve_ops.CUSTOM_DVE_SPECS[op.name] = op.spec
    dve_ops._SUB_OPCODE_FOR_NAME[op.name] = (
        dve_ops._CUSTOM_DVE_ROW_BASE + len(dve_ops.OPS) - 1)
    return op


def _build_module(reps=1, mode="full"):
    import concourse.bacc as bacc
    import concourse.mybir as mybir
    import concourse.tile as tile
    from concourse.bass import ds, ts
    from concourse.masks import make_identity

    PUN16 = _register_pun()

    f32 = mybir.dt.float32
    bf16 = mybir.dt.bfloat16
    i16 = mybir.dt.int16
    AF = mybir.ActivationFunctionType
    MUL = mybir.AluOpType.mult
    ADD = mybir.AluOpType.add

    nc = bacc.Bacc("TRN2", target_bir_lowering=False, debug=False,
                   num_devices=NCORES)

    x_d = nc.dram_tensor("x", [N, C], f32, kind="ExternalInput")
    bias_d = nc.dram_tensor("atten_bias", [N, N], f32, kind="ExternalInput")
    wqkv_d = nc.dram_tensor("w_qkv", [3 * C, C], f32, kind="ExternalInput")
    wproj_d = nc.dram_tensor("w_proj", [C, C], f32, kind="ExternalInput")
    bproj_d = nc.dram_tensor("b_proj", [C], f32, kind="ExternalInput")
    y_d = nc.dram_tensor("y", [N, C], f32, kind="ExternalOutput")

    with tile.TileContext(nc) as tc:
      for _rep in range(reps):
            with (
                tc.tile_pool(name="const", bufs=1) as const,
                tc.tile_pool(name="big", bufs=1) as big,
                tc.tile_pool(name="epool", bufs=3) as epool,
                tc.tile_pool(name="spool", bufs=3) as spool,
                tc.tile_pool(name="sigpool", bufs=2) as sigpool,
                tc.tile_pool(name="otn", bufs=1) as otn,
                tc.tile_pool(name="zstage", bufs=1) as zstage,
                tc.tile_pool(name="rzpool", bufs=2) as rzpool,
                tc.tile_pool(name="ytpool", bufs=2) as ytpool,
                tc.tile_pool(name="ystage", bufs=3) as ystage,
                tc.tile_pool(name="bstage", bufs=2) as bstage,
                tc.tile_pool(name="dpool", bufs=2, space="DRAM") as dpool,
                tc.tile_pool(name="aux", bufs=2, space="PSUM") as aux,
                tc.tile_pool(name="scps", bufs=2, space="PSUM") as scps,
                tc.tile_pool(name="outps", bufs=2, space="PSUM") as outps,
            ):
                ident = const.tile([128, 128], f32)
                make_identity(nc, ident)
                # Z-broadcast selector: out[m,n] = Z(row 32) for m<64,
                # Z(row 96) for m>=64, via one matmul per b-half.
                zsel = const.tile([128, 128], f32)
                nc.vector.memset(zsel, 0.0)
                nc.vector.memset(zsel[32:33, 0:64], 1.0)
                nc.vector.memset(zsel[96:97, 64:128], 1.0)
                bproj_sb = const.tile([128, 2], f32)
                nc.sync.dma_start(bproj_sb, bproj_d[:].rearrange("(j p) -> p j", p=128))

                wqkvT = const.tile([128, 2, 768], bf16)   # [c, cc, o]
                wprojT = const.tile([128, 2, 256], bf16)  # [c, cc, j]
                wprojP = const.tile([128, 4, 256], bf16)  # permuted for aug layout
                qT = big.tile([128, HG, N], bf16)         # [32h+d, hg, n]
                kT = big.tile([128, HG, N], bf16)
                v_aug = big.tile([128, MT, 8, 33], bf16)  # [m, mt, h, (d|1)]
                outTn = otn.tile([128, 4, N], bf16)       # [aug-c, 2hg+b, n]
                sig_tiles = {}                            # nbi -> [m, mt, 512]

                # ---------------- P0: weights + x transpose + qkv ----------------
                def prologue(stage, xtp):
                    xT = xtp.tile([128, 2, N], bf16)      # [c, cc, n]
                    # w_qkv^T and w_proj^T
                    for wt, (wd, rows) in enumerate([(wqkv_d, 6), (wproj_d, 2)]):
                        dest = wqkvT if wt == 0 else wprojT
                        for ot in range(rows):
                            wst = stage.tile([128, 256], f32, tag="wst")
                            nc.sync.dma_start(wst, wd[ts(ot, 128), :])
                            pst = aux.tile([128, 512], f32, tag="aux")
                            for cc in range(2):
                                nc.tensor.transpose(pst[:, ts(cc, 128)],
                                                    wst[:, ts(cc, 128)], ident)
                            for cc in range(2):
                                nc.scalar.copy(dest[:, cc, ts(ot, 128)],
                                               pst[:, ts(cc, 128)])
                    # x^T
                    for nt in range(MT):
                        xst = stage.tile([128, 256], f32, tag="xst")
                        nc.sync.dma_start(xst, x_d[ts(nt, 128), :])
                        pst = aux.tile([128, 512], f32, tag="aux")
                        for cc in range(2):
                            nc.tensor.transpose(pst[:, ts(cc, 128)],
                                                xst[:, ts(cc, 128)], ident)
                        for cc in range(2):
                            nc.vector.tensor_copy(xT[:, cc, ts(nt, 128)],
                                                  pst[:, ts(cc, 128)])

                    # qT, kT  (o tiles 0,1 -> q ; 2,3 -> k); q scaled by ALPHA.
                    # nb-major emission so attention on (nb=0, hg=0) can start
                    # as soon as its two projections land.
                    def qk(og, nb):
                        dest = qT if og < 2 else kT
                        hg = og % 2
                        ps = aux.tile([128, 512], f32, tag="aux")
                        for cc in range(2):
                            nc.tensor.matmul(ps, wqkvT[:, cc, ts(og, 128)],
                                             xT[:, cc, ts(nb, 512)],
                                             start=(cc == 0), stop=(cc == 1))
                        if og < 2:
                            nc.vector.tensor_scalar_mul(
                                dest[:, hg, ts(nb, 512)], ps, ALPHA)
                        else:
                            nc.vector.tensor_copy(dest[:, hg, ts(nb, 512)], ps)

                    def v_tile(mt):
                        ps = aux.tile([128, 512], f32, tag="aux")
                        for cc in range(2):
                            nc.tensor.matmul(ps[:, :256], xT[:, cc, ts(mt, 128)],
                                             wqkvT[:, cc, 512:768],
                                             start=(cc == 0), stop=(cc == 1))
                        nc.vector.tensor_copy(
                            v_aug[:, mt, :, 0:32],
                            ps[:, :256].rearrange("p (h d) -> p h d", h=8))

                    nc.vector.memset(v_aug[:, :, :, 32:33], 1.0)
                    qk(0, 0)
                    qk(2, 0)
                    qk(1, 0)
                    qk(3, 0)
                    for mt in range(MT):
                        v_tile(mt)
                    for nb in range(1, NB):
                        for og in [0, 2, 1, 3]:
                            qk(og, nb)
                    # permuted w_proj^T matching the [out|Z] interleaved layout:
                    # chunk cc2 = 2*hg + b holds head (4hg+2b) at rows 0-31 and
                    # head (4hg+2b+1) at rows 64-95; Z rows get zero weights.
                    nc.vector.memset(wprojP, 0.0)
                    for hg in range(HG):
                        for b in range(2):
                            nc.sync.dma_start(wprojP[0:32, 2 * hg + b, :],
                                              wprojT[64 * b:64 * b + 32, hg, :])
                            nc.sync.dma_start(wprojP[64:96, 2 * hg + b, :],
                                              wprojT[64 * b + 32:64 * b + 64, hg, :])

                with tc.tile_pool(name="stage", bufs=4) as stage, \
                     tc.tile_pool(name="xtp", bufs=1) as xtp:
                    prologue(stage, xtp)

                # ---------------- P1+P2: attention, bias sigmoid JIT -------------
                def bias_unit(sigT, bst, nt4, mq, on_dve):
                    pst = aux.tile([128, 512], f32, tag="aux")
                    for j in range(4):
                        nc.tensor.transpose(
                            pst[:, ts(j, 128)],
                            bst[:, mq, ts(j, 128)], ident)
                    sg = sigT[:, mq * 4:(mq + 1) * 4, ts(nt4, 128)]
                    nc.scalar.activation(
                        sg, pst.rearrange("p (j f) -> p j f", j=4),
                        AF.Tanh, scale=0.5)
                    if on_dve:
                        nc.vector.tensor_scalar(sg, sg, 1.0, 1.0, MUL, ADD)
                    else:
                        nc.gpsimd.tensor_scalar(sg, sg, 1.0, 1.0, MUL, ADD)

                def bias_block(nbi, on_dve=False):
                    # produce sig tile (=2*sigmoid^T) for bias rows
                    # [512*nbi, 512*nbi+512); one fully-contiguous 1MB DMA
                    # per 128 bias rows (cheap SP dispatch), then per-unit
                    # closures the caller spreads between attention steps.
                    sigT = sigpool.tile([128, MT, 512], bf16, tag="sig")
                    sig_tiles[nbi] = sigT
                    units = []
                    for nt4 in range(4):
                        nt = 4 * nbi + nt4

                        def load(nt=nt):
                            bst = bstage.tile([128, 4, 512], f32, tag="bst")
                            nc.sync.dma_start(
                                bst, bias_d[ts(nt, 128), :].rearrange(
                                    "p (a f) -> p a f", a=4))
                            return bst

                        units.append((load, nt4))
                    out = []
                    for load, nt4 in units:
                        def group(load=load, nt4=nt4):
                            bst = load()
                            for mq in range(4):
                                bias_unit(sigT, bst, nt4, mq, on_dve)
                        out.append(group)
                    return out

                LAG = 6  # j-steps between scores+mul and the matching attnv
                pending = []

                def drain(limit):
                    while len(pending) > limit:
                        pending.pop(0)()

                do_mul = mode != "scores"
                do_attnv = mode in ("full", "noexp")
                do_exp = mode in ("full",)
                for u in bias_block(0, on_dve=True):
                    u()
                bias_units = []
                for nb in range(NB):
                    if nb + 1 < NB:
                        bias_units = bias_block(nb + 1)
                    for hg in range(HG):
                        out_ab = [outps.tile([128, 512], f32, tag="o",
                                             name=f"oab{hg}{nb}{b}")
                                  for b in range(2)]
                        e_ts = [None] * 4

                        def attnv(mq, j, hg=hg, nb=nb, out_ab=out_ab, e_ts=e_ts):
                            mt = mq * 4 + j
                            first = mt == 0
                            last = mt == MT - 1
                            e_t = e_ts[mq % 4]
                            for h in range(4):
                                nc.tensor.matmul(
                                    out_ab[h // 2][64 * (h % 2):64 * (h % 2) + 33, :],
                                    v_aug[:, mt, hg * 4 + h, :],
                                    e_t[:, j, h],
                                    start=first, stop=last,
                                    tile_position=(0, 64 * (h % 2)))

                        def tail(hg=hg, nb=nb, out_ab=out_ab):
                            # stage out+Z, broadcast Z across partitions with a
                            # select-matmul (no DRAM round-trip), then divide
                            # on the Pool engine (its only PSUM-free job).
                            st = zstage.tile([128, 2, 512], f32, tag="zst")
                            for b in range(2):
                                nc.scalar.copy(st[:, b, :], out_ab[b])
                            for b in range(2):
                                zb_ps = aux.tile([128, 512], f32, tag="aux")
                                nc.tensor.matmul(zb_ps, zsel, st[:, b, :],
                                                 start=True, stop=True)
                                zb = rzpool.tile([128, 512], f32, tag="rz")
                                nc.scalar.copy(zb, zb_ps)
                                rzb = rzpool.tile([128, 512], f32, tag="rz2")
                                nc.vector.reciprocal_approx_fast(rzb, zb)
                                nc.gpsimd.tensor_tensor(
                                    outTn[:, 2 * hg + b, ts(nb, 512)],
                                    st[:, b, :], rzb, MUL)
                            if hg == 1:
                                yts = []
                                for jt in range(2):
                                    pp = aux.tile([128, 512], f32, tag="aux")
                                    for cc2 in range(4):
                                        nc.tensor.matmul(
                                            pp, wprojP[:, cc2, ts(jt, 128)],
                                            outTn[:, cc2, ts(nb, 512)],
                                            start=(cc2 == 0), stop=(cc2 == 3))
                                    yt = ytpool.tile([128, 512], f32, tag="yt")
                                    nc.scalar.activation(
                                        yt, pp, AF.Identity,
                                        bias=bproj_sb[:, jt:jt + 1])
                                    yts.append(yt)
                                for k in range(4):
                                    nt = nb * 4 + k
                                    yo = aux.tile([128, 512], f32, tag="aux")
                                    for jt in range(2):
                                        nc.tensor.transpose(
                                            yo[:, ts(jt, 128)],
                                            yts[jt][:, ts(k, 128)], ident)
                                    y_st = ystage.tile([128, 256], f32, tag="yst")
                                    nc.scalar.copy(y_st, yo[:, :256])
                                    nc.sync.dma_start(y_d[ts(nt, 128), :], y_st)

                        def j_step(mq, j, e_t, hg=hg, nb=nb):
                            mt = mq * 4 + j
                            exp_hps = []
                            for hp in range(2):
                                punned = ((mt, hp) in PUN_SLOTS and do_exp
                                          and do_mul)
                                sc = scps.tile([128, 2, 512], f32, tag="s")
                                for hh in range(2):
                                    h = hp * 2 + hh
                                    nc.tensor.matmul(
                                        sc[:, hh, :],
                                        kT[32 * h:32 * (h + 1), hg, ts(mt, 128)],
                                        qT[32 * h:32 * (h + 1), hg, ts(nb, 512)],
                                        start=True, stop=True,
                                        tile_position=(32 * h, 0))
                                sig_bc = sig_tiles[nb][:, mt:mt + 1,
                                                       :].to_broadcast(
                                                           (128, 2, 512))
                                dst = e_t[:, j, 2 * hp:2 * hp + 2]
                                if not do_mul:
                                    nc.vector.tensor_copy(dst, sc)
                                elif punned:
                                    nc.vector._custom_dve(
                                        PUN16, out=dst.bitcast(i16),
                                        in0=sig_bc, in1=sc[:, :, :],
                                        s0=MAGIC + OFF, s1=MAGIC)
                                else:
                                    if (mt, hp) in PSI_SLOTS:
                                        s_sb = spool.tile([128, 2, 512], bf16,
                                                          tag="ssb")
                                        nc.scalar.copy(s_sb, sc)
                                        nc.vector.tensor_tensor(
                                            dst, s_sb, sig_bc, MUL)
                                    else:
                                        nc.vector.tensor_tensor(
                                            dst, sc, sig_bc, MUL)
                                    exp_hps.append(hp)
                            if do_exp and exp_hps:
                                if len(exp_hps) == 2:
                                    nc.scalar.activation(
                                        e_t[:, j], e_t[:, j], AF.Exp,
                                        scale=EXPSCALE)
                                else:
                                    hp = exp_hps[0]
                                    sl = e_t[:, j, 2 * hp:2 * hp + 2]
                                    nc.scalar.activation(
                                        sl, sl, AF.Exp, scale=EXPSCALE)

                        for mq in range(4):
                            e_t = epool.tile([128, 4, 4, 512], bf16, tag="e")
                            e_ts[mq % 4] = e_t
                            for j in range(4):
                                # drain one pending attnv BEFORE emitting the
                                # next scores so the PE never head-of-line
                                # blocks on the extract semaphore while attnv
                                # work is available.
                                if do_attnv:
                                    drain(LAG)
                                j_step(mq, j, e_t)
                                if do_attnv:
                                    pending.append(
                                        (lambda mq=mq, j=j, fn=attnv:
                                         fn(mq, j)))
                            if bias_units and (mq % 2 == 1 or hg == 1):
                                bias_units.pop(0)()
                        if do_attnv:
                            pending.append(tail)
                drain(0)

    nc.compile()
    return nc


def _get_module():
    if "nc" not in _cache:
        _cache["nc"] = _build_module()
    return _cache["nc"]


class _Runner:
    """Persistent jitted shard_map executor (mirrors bass2jax.run_bass_via_pjrt
    but keeps one jit cache entry so repeated calls don't recompile)."""

    def __init__(self, nc):
        import jax
        from jax.experimental.shard_map import shard_map
        from jax.sharding import Mesh, NamedSharding, PartitionSpec

        import concourse.mybir as mybir
        from concourse import bass2jax

        bass2jax.install_neuronx_cc_hook()
        assert nc.dbg_addr is None
        partition_name = (nc.partition_id_tensor.name
                          if nc.partition_id_tensor else None)
        in_names, out_names, out_avals, zero_outs = [], [], [], []
        for alloc in nc.m.functions[0].allocations:
            if not isinstance(alloc, mybir.MemoryLocationSet):
                continue
            name = alloc.memorylocations[0].name
            if alloc.kind == "ExternalInput":
                if name != partition_name:
                    in_names.append(name)
            elif alloc.kind == "ExternalOutput":
                out_names.append(name)
                shape = tuple(alloc.tensor_shape)
                dtype = mybir.dt.np(alloc.dtype)
                out_avals.append(jax.core.ShapedArray(shape, dtype))
                zero_outs.append(np.zeros(shape, dtype))
        self.in_names = in_names
        self.out_names = out_names
        self.out_avals = out_avals
        all_in = tuple(in_names) + tuple(out_names)
        if partition_name is not None:
            all_in = all_in + (partition_name,)

        def _body(*args):
            operands = list(args)
            if partition_name is not None:
                operands.append(bass2jax.partition_id_tensor())
            outs = bass2jax._bass_exec_p.bind(
                *operands,
                out_avals=tuple(out_avals),
                in_names=all_in,
                out_names=tuple(out_names),
                lowering_input_output_aliases=(),
                sim_require_finite=True,
                sim_require_nnan=True,
                nc=nc,
            )
            return tuple(outs)

        devices = jax.devices()[:NCORES]
        mesh = Mesh(np.asarray(devices), ("core",))
        nspec = len(in_names) + len(out_names)
        self._fn = jax.jit(
            shard_map(_body, mesh=mesh,
                      in_specs=(PartitionSpec("core"),) * nspec,
                      out_specs=(PartitionSpec("core"),) * len(out_names),
                      check_rep=False),
            keep_unused=True)
        self._sharding = NamedSharding(mesh, PartitionSpec("core"))
        self._jax = jax
        self._zero_dev = [
            jax.device_put(np.concatenate([z] * NCORES, axis=0), self._sharding)
            for z in zero_outs
        ]

    def put_inputs(self, in_maps):
        concat = [
            np.concatenate([np.asarray(m[nm]) for m in in_maps], axis=0)
            for nm in self.in_names
        ]
        return [self._jax.device_put(a, self._sharding) for a in concat]

    def run(self, dev_inputs):
        outs = self._fn(*dev_inputs, *self._zero_dev)
        self._jax.block_until_ready(outs)
        return outs


def _get_runner():
    if "runner" not in _cache:
        _cache["runner"] = _Runner(_get_module())
    return _cache["runner"]


def _make_in_maps(x, atten_bias, w_qkv, w_proj, b_proj):
    x = np.asarray(x, dtype=np.float32)
    atten_bias = np.ascontiguousarray(np.asarray(atten_bias, dtype=np.float32))
    w_qkv = np.ascontiguousarray(np.asarray(w_qkv, dtype=np.float32))
    w_proj = np.ascontiguousarray(np.asarray(w_proj, dtype=np.float32))
    b_proj = np.ascontiguousarray(np.asarray(b_proj, dtype=np.float32))
    return [
        {
            "x": np.ascontiguousarray(x[b]),
            "atten_bias": atten_bias,
            "w_qkv": w_qkv,
            "w_proj": w_proj,
            "b_proj": b_proj,
        }
        for b in range(B)
    ]


def kernel(x, atten_bias, w_qkv, w_proj, b_proj):
    runner = _get_runner()
    in_maps = _make_in_maps(x, atten_bias, w_qkv, w_proj, b_proj)
    dev = runner.put_inputs(in_maps)
    outs = runner.run(dev)
    y = np.asarray(outs[runner.out_names.index("y")])
    return y.reshape(B, N, C).astype(np.float32)
